# revision 20
# baseline (speedup 1.0000x reference)
"""MatchNet retrieval-KNN kernel for 8 Trainium2 NeuronCores.

Strategy (candidate-sharded fp8 device pass + exact host re-score):
  Host:  A = W^T W = V diag(lam) V^T.  Drop the smallest eigendirection:
         F = V[:, :255] sqrt(lam)  ->  s(q,n) ~= (F^T x_q).(F^T c_n) - |c|_A^2/2
         with truncation error ~lam_min (negligible vs fp8 noise).  The norm
         term rides along as a 256th contraction coordinate (query side 128,
         candidate side 2*gc), so the device runs ONE fp8 DoubleRow matmul
         (K=256) per [128q x 512n] tile -- no bias matmuls.
  Device (per core, 12800 padded candidates, 8 query tiles):
         PE: 25 DR matmuls/qtile into 2-bank PSUM pairs.
         ACT: PSUM fp32 -> SBUF fp16 scores (x1/256 scale), batched copies.
         Pool(gpsimd): tree L1 = max of 8-halves (16 -> 8 per window).
         DVE: tree L2..L4 -> per-16-window maxima [128, 800]; per 1600-cand
         segment: max8 (top-8 window values), max_index on L4 (window id),
         max_index on L2 (position of the value at 4-candidate resolution).
  Host:  merge 8x64 entries/row, take top-192 by value, exact re-score the
         4 candidates of each entry (fp32 BLAS, fp64 refine of top-48), exact
         top-32 softmax.  find_index8 assigns duplicate values sequential
         positions, so ties are safe; window-id vs position mismatches (rare
         cross-window value collisions) fall back to re-scoring the full
         16-candidate window.

Toolchain note: walrus rejects >1 sync wait per instruction; _legalize_waits
peels extra waits onto single-wait same-engine NoOps in the BIR JSON.
"""

import json
import os
import types

import ml_dtypes
import numpy as np

import concourse.bass as bass
import concourse.mybir as mybir
import concourse.tile as tile
from concourse.bass import ds
from concourse.bass_utils import run_bass_kernel_spmd

B, N, D_IN, DIM, NUMK = 1024, 100000, 256, 512, 32
TEMP = 1.0
NCORES = 8
NSHARD_REAL = N // NCORES   # 12500
NSHARD = 12800              # padded per-core candidate count
QT = B // 128               # 8 query tiles
NT = NSHARD // 512          # 25 psum tiles per qtile
NWIN = NSHARD // 16         # 800 16-candidate windows
SEG_C = 1600                # candidates per segment
SEGS = NSHARD // SEG_C      # 8 segments per qtile row
SEG_W = SEG_C // 16         # 100 windows per segment
ENT = SEGS * 8              # 64 entries per (row, core)
K_SAFE = 192                # host re-scores this many entries per row
SCALE = 16.0                # fp8 input scale (score arrives x256, ACT /256)

F32 = mybir.dt.float32
F16 = mybir.dt.float16
FP8 = mybir.dt.float8e4
U16 = mybir.dt.uint16
ACT_COPY = mybir.ActivationFunctionType.Copy
MAX = mybir.AluOpType.max
DR = mybir.MatmulPerfMode.DoubleRow


def _legalize_waits(nc):
    """Wrap nc.to_json_bytes so every instruction carries <=1 sync wait."""
    orig = nc.to_json_bytes

    def patched(self):
        m = json.loads(orig())
        ctr = 0
        for fn in m["functions"]:
            for blk in fn["blocks"]:
                out = []
                for inst in blk["instructions"]:
                    si = inst.get("sync_info")
                    waits = (si or {}).get("on_wait") or []
                    if len(waits) > 1:
                        for w in waits[:-1]:
                            ctr += 1
                            out.append({
                                "debug": inst.get("debug", 0),
                                "engine": inst["engine"],
                                "ins": [],
                                "name": f"I-nopw{ctr}",
                                "opcode": "NoOp",
                                "outs": [],
                                "sync_info": {"on_wait": [w],
                                              "on_update": []},
                            })
                        si["on_wait"] = waits[-1:]
                    out.append(inst)
                blk["instructions"] = out
        return json.dumps(m).encode()

    nc.to_json_bytes = types.MethodType(patched, nc)
    return nc


def _build_bass():
    nc = bass.Bass()
    xa_d = nc.dram_tensor("xa", [128, 2 * B], FP8, kind="ExternalInput")
    cx_d = nc.dram_tensor("cx", [128, 2 * NSHARD], FP8, kind="ExternalInput")
    ol2_d = nc.dram_tensor("ol2", [B, NSHARD // 4], F16,
                           kind="ExternalOutput")

    with (
        tile.TileContext(nc) as tc,
        tc.tile_pool(name="const", bufs=1) as constp,
        tc.tile_pool(name="sc", bufs=3) as scp,
        tc.tile_pool(name="tr", bufs=3) as trp,
        tc.tile_pool(name="ps", bufs=3, space="PSUM") as psp,
        tc.tile_pool(name="ps1", bufs=2, space="PSUM") as psp1,
    ):
        xa_sb = constp.tile([128, 2, B], FP8)
        nc.sync.dma_start(xa_sb, xa_d.rearrange("p (t q) -> p t q", q=B))
        cx_sb = constp.tile([128, 2, NSHARD], FP8)
        cx_view = cx_d.rearrange("p (t n) -> p t n", n=NSHARD)
        for ch in range(5):
            nc.sync.dma_start(
                cx_sb[:, :, ds(ch * 2560, 2560)],
                cx_view[:, :, ds(ch * 2560, 2560)])

        for q in range(QT):
            lhsT = xa_sb[:, :, ds(q * 128, 128)]
            scores = scp.tile([128, NWIN, 16], F16, name="scores")
            sflat = scores.rearrange("p a b -> p (a b)")
            for nt2 in range(NT // 2):
                ps = psp.tile([128, 2, 512], F32)
                for h in range(2):
                    nt = nt2 * 2 + h
                    nc.tensor.matmul(
                        ps[:, h], lhsT,
                        cx_sb[:, :, ds(nt * 512, 512)],
                        start=True, stop=True, perf_mode=DR)
                if nt2 < 9:
                    nc.scalar.activation(
                        sflat[:, ds(nt2 * 1024, 1024)],
                        ps.rearrange("p a b -> p (a b)"), ACT_COPY)
                else:
                    nc.vector.tensor_copy(
                        sflat[:, ds(nt2 * 1024, 1024)],
                        ps.rearrange("p a b -> p (a b)"))
            # odd 25th tile
            ps1 = psp1.tile([128, 512], F32)
            nc.tensor.matmul(
                ps1, lhsT, cx_sb[:, :, ds((NT - 1) * 512, 512)],
                start=True, stop=True, perf_mode=DR)
            nc.scalar.activation(
                sflat[:, ds((NT - 1) * 512, 512)], ps1, ACT_COPY)

            # comb-max tree: 16 -> 8 -> 4 (stride-4 combs of each 16-window)
            l1 = trp.tile([128, NWIN, 8], F16, name="l1")
            nc.vector.tensor_tensor(
                out=l1, in0=scores[:, :, 0:8], in1=scores[:, :, 8:16], op=MAX)
            l2 = trp.tile([128, NWIN, 4], F16, name="l2")
            nc.vector.tensor_tensor(
                out=l2, in0=l1[:, :, 0:4], in1=l1[:, :, 4:8], op=MAX)
            nc.gpsimd.dma_start(
                ol2_d[ds(q * 128, 128), :].rearrange(
                    "p (a b) -> p a b", b=4), l2)
    return _legalize_waits(nc)


_NC_CACHE = {}


def kernel(x, candidate_x, candidate_y, W, b, context_size, is_train):
    x = np.asarray(x, dtype=np.float32)
    candidate_x = np.asarray(candidate_x, dtype=np.float32)
    candidate_y = np.asarray(candidate_y, dtype=np.float32)
    W = np.asarray(W, dtype=np.float32)
    b = np.asarray(b, dtype=np.float32)

    A = (W.T @ W).astype(np.float32)              # [256, 256]
    lam, V = np.linalg.eigh(A.astype(np.float64))  # ascending
    F = (V[:, 1:] * np.sqrt(lam[1:])).astype(np.float32)  # [256, 255]
    xP = F.T @ x.T                                 # [255, 1024]
    cP = F.T @ candidate_x.T                       # [255, 100000]
    Z = candidate_x @ A                            # [N, 256] (reused exact)
    cn2 = np.einsum("ij,ij->i", candidate_x, Z)    # c^T A c
    gc = -0.5 * cn2
    gmean = float(gc.mean())
    gcc = (gc - gmean).astype(np.float32)          # centered; rank-invariant

    # device matrices: 255 projected dims + norm coordinate
    Xd = np.zeros((256, B), dtype=np.float32)
    Xd[:255] = np.clip(SCALE * xP, -240, 240)
    Xd[255] = 128.0
    xa8 = np.ascontiguousarray(
        Xd.reshape(2, 128, B).transpose(1, 0, 2).reshape(128, 2 * B)
    ).astype(ml_dtypes.float8_e4m3)

    in_maps = []
    for c in range(NCORES):
        Cd = np.zeros((256, NSHARD), dtype=np.float32)
        sl = slice(c * NSHARD_REAL, (c + 1) * NSHARD_REAL)
        Cd[:255, :NSHARD_REAL] = np.clip(SCALE * cP[:, sl], -240, 240)
        Cd[255, :NSHARD_REAL] = np.clip(2.0 * gcc[sl], -240, 240)
        Cd[255, NSHARD_REAL:] = -240.0             # pads sink to the bottom
        Cd *= 1.0 / 256.0                          # fp16 scores in raw units
        cx8 = np.ascontiguousarray(
            Cd.reshape(2, 128, NSHARD).transpose(1, 0, 2).reshape(
                128, 2 * NSHARD)).astype(ml_dtypes.float8_e4m3)
        in_maps.append({"xa": xa8, "cx": cx8})

    if "nc" not in _NC_CACHE:
        _NC_CACHE["nc"] = _build_bass()
    nc = _NC_CACHE["nc"]

    trace = bool(int(os.environ.get("KERNEL_TRACE", "0")))
    res = run_bass_kernel_spmd(nc, in_maps, core_ids=list(range(NCORES)),
                               trace=trace)
    if trace:
        print(f"HW exec time: {res.exec_time_ns} ns")
        print(f"mean exec time: {res.mean_exec_time_ns} ns")
        if res.instructions_and_trace is not None:
            print("trace:", res.instructions_and_trace[1])

    # ---- host merge: global top-K_SAFE combs from the L2 pyramids ----
    NC4 = NSHARD // 4                                  # 3200 combs per core
    vals = np.concatenate(
        [r["ol2"].astype(np.float32) for r in res.results], axis=1)
    # [B, 8*3200]; col c*3200 + w*4 + i = comb {w*16 + i + 4k} of core c

    rows = np.arange(B)[:, None]
    sel = np.argpartition(-vals, K_SAFE, axis=1)[:, :K_SAFE]       # [B,192]
    sv_core = sel // NC4
    sv_col = sel % NC4
    pad_base = (sv_col // 4) * 16 + sv_col % 4
    cand4 = pad_base[:, :, None] + 4 * np.arange(4)[None, None, :]  # [B,192,4]
    ok = cand4 < NSHARD_REAL
    cand4 = sv_core[:, :, None] * NSHARD_REAL + np.clip(
        cand4, 0, NSHARD_REAL - 1)
    cand = cand4.reshape(B, K_SAFE * 4)
    valid = ok.reshape(B, K_SAFE * 4)

    # stage 1: fp32 exact scores s' = (xA).c - cn2/2 for all selected
    xA = (x @ A).astype(np.float32)                                # [B, 256]
    C_sel = candidate_x[cand]                                      # [B,768,256]
    s1 = np.matmul(C_sel, xA[:, :, None])[:, :, 0] - 0.5 * cn2[cand]
    s1[~valid] = -np.inf

    # stage 2: fp64 refine of top-48
    NF = 48
    top1 = np.argpartition(-s1, NF, axis=1)[:, :NF]                # [B,48]
    cand_f = cand[rows, top1]
    s_exact = (np.einsum("rd,rkd->rk", xA.astype(np.float64),
                         candidate_x[cand_f].astype(np.float64))
               - 0.5 * cn2[cand_f])

    # sort by exact score, dedup repeated candidate ids, take top-33
    ordK = np.argsort(-s_exact, axis=1, kind="stable")
    cand_sorted = np.take_along_axis(cand_f, ordK, axis=1)
    s_sorted = np.take_along_axis(s_exact, ordK, axis=1)
    K_top = NUMK + 1
    top = np.zeros((B, K_top), dtype=np.int64)
    s_top = np.full((B, K_top), -np.inf)
    for r in range(B):
        ids_r = cand_sorted[r]
        _, first_idx = np.unique(ids_r, return_index=True)
        keep = np.zeros(len(ids_r), dtype=bool)
        keep[first_idx] = True
        kk = np.nonzero(keep)[0][:K_top]
        top[r, :len(kk)] = ids_r[kk]
        s_top[r, :len(kk)] = s_sorted[r][kk]

    cand_sel = top[:, :NUMK]
    s_sel = s_top[:, :NUMK]
    xe = (x @ W.T + b).astype(np.float32)
    xn2 = np.sum(xe.astype(np.float64) ** 2, axis=1)
    const_q = x.astype(np.float64) @ (W.T @ b).astype(np.float64) \
        + 0.5 * float(b.astype(np.float64) @ b.astype(np.float64))

    d2 = xn2[:, None] - 2.0 * (s_sel + const_q[:, None])
    d = np.sqrt(np.maximum(d2, 0.0)) / TEMP
    neg = -d
    neg -= neg.max(axis=1, keepdims=True)
    wgt = np.exp(neg)
    wgt /= wgt.sum(axis=1, keepdims=True)
    logits = np.sum(wgt * candidate_y[cand_sel].astype(np.float64), axis=1)

    # Rows whose rank-32/33 gap is within fp32 rounding ambiguity: re-rank
    # with reference-style fp32 arithmetic so the boundary pick matches.
    gap32 = s_top[:, NUMK - 1] - s_top[:, NUMK]
    for r in np.where(gap32 < 0.01)[0]:
        ids_r = cand_sorted[r]
        _, first_idx = np.unique(ids_r, return_index=True)
        keep = np.sort(first_idx)
        csel = ids_r[keep]
        ce_sel = (candidate_x[csel] @ W.T + b).astype(np.float32)
        sq = (np.sum(xe[r] ** 2, dtype=np.float32)
              + np.sum(ce_sel ** 2, axis=1, dtype=np.float32)
              - 2.0 * (ce_sel @ xe[r]))
        d_r = np.sqrt(np.maximum(sq, 0.0)) / TEMP
        o32 = np.argsort(d_r, kind="stable")[:NUMK]
        nb = (-d_r[o32]).astype(np.float64)
        nb -= nb.max()
        wr = np.exp(nb)
        wr /= wr.sum()
        logits[r] = float(wr @ candidate_y[csel[o32]].astype(np.float64))
    return logits.astype(np.float32)
